# revision 28
# baseline (speedup 1.0000x reference)
"""Hadamard transform kernel for Trainium2 (8 NeuronCores, SPMD data parallel).

y = (1/48) * (H36 (x) H64) @ x_row  per token row, x: (4, 8192, 2304) fp32.

Math: view each row as X[j=36, c=64] (row-major).  Then
    y[k*64+m] = (1/48) * sum_j sum_c had_k[k,j] * H64[m,c] * X[j,c]
with H64 the natural-order Sylvester Hadamard (symmetric).

Device scheme (per 6-token "group"):
  mm1 (data-stationary, one per group):
       lhsT = Xg[(t3,j)=108 part, (trip2,c)=128 free]
       rhs  = W36 = blockdiag(had_k.T x3)/48 [108,108]
       out  = Z[(trip2,c)=128, (t3,k)=108]  (PSUM fp32)
  mm2 (weights-stationary, one per QUAD of 4 groups):
       lhsT = W64 = blockdiag(H64 x2) [128,128]  (reloaded rarely)
       rhs  = Z quad [(trip2,c)=128 part, (4g,t3,k)=432 free]  (fp16 SBUF)
       out  = Y^T [(trip2,m)=128 part, (4g,t3,k)=432 free]  (PSUM, one bank)
  The transposed Y layout is free: the host un-permutes whatever layout
  the kernel stores.

HBM layout: the HOST pre-permutes x into the exact SBUF tile layout
(fp16), so every DMA is a fully contiguous 442 KB transfer with 4 KB
per-partition descriptors (a scatter-AP kernel runs at ~80-125 GB/s due
to 128/256-byte descriptors; contiguous runs near HBM rate).  The host
un-permutes the fp16 output back to token-major fp32.

Per core: 4096 tokens padded to 4128 = 43 superblocks x 16 groups x 6.
PSUM->SBUF copies batch 8 groups (one "oct"): z-copy on DVE, y-copy on
ACT.  The PE program is software-pipelined 2 octs deep (mm1 of oct i+2
before mm2 of oct i) so copy latency stays off the PE critical path.
DMA-completion semaphores are per buffer slot: the 16 SDMA engines'
increments from back-to-back DMAs interleave, so a shared counter
cannot prove any single transfer finished.
"""

import numpy as np

D = 2304
NTOK = 4096            # real tokens per core
NCORES = 8
GP_SB = 16             # groups per superblock (one DMA)
OCT = 8                # groups per PSUM batch
OPS = GP_SB // OCT     # octs per superblock = 2
NSB = 43               # superblocks per core
NGRP = NSB * GP_SB     # 688 groups = 4128 tokens (32 pad)
NTOKP = NGRP * 6       # 4128
FREE_SB = GP_SB * 128  # x elems per partition per superblock
YFREE = 108            # y free elems per group per partition
YFREE_SB = GP_SB * YFREE


def _h64():
    m, c = np.meshgrid(np.arange(64), np.arange(64), indexing="ij")
    bits = np.zeros((64, 64), np.int64)
    v = m & c
    for _ in range(6):
        bits += v & 1
        v >>= 1
    return np.where(bits % 2 == 0, 1.0, -1.0).astype(np.float32)


def _build_program(wboth_np):
    from contextlib import ExitStack
    import concourse.bass as bass
    import concourse.mybir as mybir
    from concourse.bass_types import AP

    nc = bass.Bass()
    x = nc.dram_tensor("x", [NSB * 108, FREE_SB], mybir.dt.float16,
                       kind="ExternalInput")
    y = nc.dram_tensor("y", [NSB * 128, YFREE_SB], mybir.dt.float16,
                       kind="ExternalOutput")
    w_d = nc.inline_tensor(wboth_np, name="wboth")

    NOCT = NSB * OPS  # 86

    # DMA "units" of two superblocks (884 KB): each transfer occupies its
    # queue ~(transfer + ~1.7 us completion receipt), so per-queue DMA
    # count — not bytes — was the limiter at one superblock per DMA.
    USB = 4                        # superblocks per DMA unit
    NU = (NSB + USB - 1) // USB    # 11 units; last unit is 3 sbs
    def u_sbs(u):
        return min(USB, NSB - USB * u)

    def u_octs(u):
        return OPS * u_sbs(u)

    def x_ap(u):
        return AP(tensor=x, offset=USB * u * 108 * FREE_SB,
                  ap=[[FREE_SB, 108], [108 * FREE_SB, u_sbs(u)], [1, FREE_SB]])

    def y_ap(u):
        return AP(tensor=y, offset=USB * u * 128 * YFREE_SB,
                  ap=[[YFREE_SB, 128], [128 * YFREE_SB, u_sbs(u)], [1, YFREE_SB]])

    with ExitStack() as ctx:
        wsb = ctx.enter_context(nc.sbuf_tensor("wsb", [128, 236], mybir.dt.float16))
        w36 = wsb[0:108, 0:108]
        w64 = wsb[:, 108:236]
        xt = [ctx.enter_context(nc.sbuf_tensor(f"xt{i}", [108, 4 * GP_SB, 128], mybir.dt.float16)) for i in range(3)]
        yt = [ctx.enter_context(nc.sbuf_tensor(f"yt{i}", [128, 4 * GP_SB, YFREE], mybir.dt.float16)) for i in range(3)]
        zsb = [ctx.enter_context(nc.sbuf_tensor(f"zsb{i}", [128, OCT, YFREE], mybir.dt.float16)) for i in range(3)]
        zps = [ctx.enter_context(nc.psum_tensor(f"zps{i}", [128, OCT, 128], mybir.dt.float32)) for i in range(2)]
        yps = [ctx.enter_context(nc.psum_tensor(f"yps{i}", [128, OCT // 4, 512], mybir.dt.float32)) for i in range(2)]
        s_in = [ctx.enter_context(nc.semaphore(name=f"s_in{i}")) for i in range(3)]
        s_out = [ctx.enter_context(nc.semaphore(name=f"s_out{i}")) for i in range(3)]
        s_pe1 = ctx.enter_context(nc.semaphore())
        s_zc = ctx.enter_context(nc.semaphore())
        s_pe2 = ctx.enter_context(nc.semaphore())
        s_yc = ctx.enter_context(nc.semaphore())
        s_w = ctx.enter_context(nc.semaphore())
        blk = ctx.enter_context(nc.Block())

        # Loads and stores each alternate between the sync (HWDGE) and
        # gpsimd (SWDGE) queues; each queue's store trails its load by two
        # units so the store's semaphore wait never delays a ready load.
        OPU = OPS * USB  # octs per full unit = 8; s_pe1 counts half-octs (2/oct)

        def emit_load(s, u):
            if u >= 3:  # xt[u%3] reuse: mm1 octs of unit u-3 done
                s.wait_ge(s_pe1, 2 * OPU * (u - 2))
            ng = u_sbs(u) * GP_SB
            s.dma_start(xt[u % 3][:, 0:ng, :],
                        x_ap(u)).then_inc(s_in[u % 3], 16)

        def emit_store(s, u):
            s.wait_ge(s_yc, min(OPU * (u + 1), NOCT))
            ng = u_sbs(u) * GP_SB
            s.dma_start(y_ap(u),
                        yt[u % 3][:, 0:ng, :]).then_inc(s_out[u % 3], 16)

        @blk.sync
        def _(s):
            s.dma_start(wsb[:, :], w_d[:, :]).then_inc(s_w, 16)
            for u in range(0, NU, 2):
                emit_load(s, u)
                if u >= 2:
                    emit_store(s, u - 2)
            for u in (NU - 2, NU - 1):
                if u >= 0 and u % 2 == 0:
                    emit_store(s, u)

        def mm1(oi):
            u = oi // OPU
            g0 = (oi % OPU) * OCT
            for q in range(OCT):
                i = nc.tensor.matmul(zps[oi % 2][:, q, 0:108],
                                     xt[u % 3][:, g0 + q, :], w36,
                                     start=(q % 4 == 0), stop=(q % 4 == 3))
                if q == 3:
                    i.then_inc(s_pe1, 1)  # first half done: z-copy can start
            i.then_inc(s_pe1, 1)

        @blk.tensor
        def _(t):
            t.wait_ge(s_w, 16)
            t.wait_ge(s_in[0], 16)
            mm1(0)
            mm1(1)
            for oi in range(NOCT):
                # 2-deep software pipeline: mm1 of oct oi+2 before mm2 of
                # oct oi so the DVE z-copy of oct oi is hidden behind a
                # whole oct of PE work.  mm1(oi+2) (zps reuse) and mm2(oi)
                # (zsb ready) gate on the same z-copy(oi) event, so two
                # zps buffers suffice.
                if oi + 2 < NOCT:
                    u2 = (oi + 2) // OPU
                    if (oi + 2) % OPU == 0:
                        t.wait_ge(s_in[u2 % 3], 16 * (u2 // 3 + 1))
                    t.wait_ge(s_zc, 2 * oi + 2)  # zps[(oi+2)%2] freed by zc(oi)
                    mm1(oi + 2)
                if oi >= 2:                # yps[oi%2] freed by y-copy of oi-2
                    t.wait_ge(s_yc, oi - 1)
                for qq in range(OCT // 4):  # one wide matmul per 4 groups
                    # wait only for this half's z-copy
                    t.wait_ge(s_zc, 2 * oi + qq + 1)
                    i = nc.tensor.matmul(yps[oi % 2][:, qq, 0:432],
                                         w64,
                                         zsb[oi % 3][:, 4 * qq:4 * qq + 4, :],
                                         start=True, stop=True)
                i.then_inc(s_pe2, 1)

        @blk.vector
        def _(v):
            for oi in range(NOCT):
                for h in range(2):
                    v.wait_ge(s_pe1, 2 * oi + h + 1)
                    if oi >= 3 and h == 0:  # zsb[oi%3] consumed by mm2 of oi-3
                        v.wait_ge(s_pe2, oi - 2)
                    nc.vector.tensor_copy(
                        zsb[oi % 3][:, 4 * h:4 * h + 4, :],
                        zps[oi % 2][:, 4 * h:4 * h + 4, 0:108]).then_inc(s_zc, 1)

        @blk.scalar
        def _(a):
            for oi in range(NOCT):
                u = oi // OPU
                o = oi % OPU
                a.wait_ge(s_pe2, oi + 1)
                if u >= 3 and o == 0:  # yt[u%3] freed by store of unit u-3
                    a.wait_ge(s_out[u % 3], 16 * (u // 3))
                nc.scalar.copy(yt[u % 3][:, o * OCT:(o + 1) * OCT, :],
                               yps[oi % 2][:, :, 0:432]).then_inc(s_yc, 1)

        @blk.gpsimd
        def _(g):
            for u in range(1, NU, 2):
                emit_load(g, u)
                if u >= 2:
                    emit_store(g, u - 2)
            for u in (NU - 2, NU - 1):
                if u >= 0 and u % 2 == 1:
                    emit_store(g, u)
    return nc


def _permute_x(x):
    """[C, NTOK, D] fp32 -> [C, NSB*108, FREE_SB] fp16 in tile layout."""
    xc = np.zeros((NCORES, NTOKP, D), np.float16)
    xc[:, :NTOK, :] = x
    xv = xc.reshape(NCORES, NSB, GP_SB, 2, 3, 36, 64)
    xr = np.ascontiguousarray(xv.transpose(0, 1, 4, 5, 2, 3, 6))
    return xr.reshape(NCORES, NSB * 108, FREE_SB)


def _unpermute_y(yr):
    """[C, NSB*128, YFREE_SB] fp16 (transposed tile layout) ->
    [C, NTOK, D] fp32."""
    yv = yr.reshape(NCORES, NSB, 2, 64, GP_SB, 3, 36)
    out32 = np.empty((NCORES, NTOKP, D), np.float32)
    out32.reshape(NCORES, NSB, GP_SB, 2, 3, 36, 64)[...] = \
        yv.transpose(0, 1, 4, 2, 5, 6, 3)
    return np.ascontiguousarray(out32[:, :NTOK])


_CACHED = {}
_LAST_RES = None


def _run(x, had_k, trace=False):
    global _LAST_RES
    from concourse.bass_utils import run_bass_kernel_spmd

    x = np.asarray(x, dtype=np.float32)
    had_k = np.asarray(had_k, dtype=np.float32)

    h64 = _h64()
    wboth_np = np.zeros((128, 236), np.float16)
    wboth_np[0:108, 0:108] = np.kron(np.eye(3, dtype=np.float32),
                                     had_k.T).astype(np.float16) / 48.0
    wboth_np[:, 108:236] = np.kron(np.eye(2, dtype=np.float32),
                                   h64).astype(np.float16)

    key = wboth_np.tobytes()
    if key not in _CACHED:
        _CACHED[key] = _build_program(wboth_np)
    nc = _CACHED[key]

    xr = _permute_x(x.reshape(NCORES, NTOK, D))
    in_maps = [{"x": xr[i]} for i in range(NCORES)]

    res = run_bass_kernel_spmd(nc, in_maps, core_ids=list(range(NCORES)),
                               trace=trace)
    _LAST_RES = res

    yr = np.stack([r["y"] for r in res.results])
    return _unpermute_y(yr).reshape(x.shape)


def kernel(x, had_k):
    return _run(x, had_k)


# revision 30
# speedup vs baseline: 1.0237x; 1.0237x over previous
"""Hadamard transform kernel for Trainium2 (8 NeuronCores, SPMD data parallel).

y = (1/48) * (H36 (x) H64) @ x_row  per token row, x: (4, 8192, 2304) fp32.

Math: view each row as X[j=36, c=64] (row-major).  Then
    y[k*64+m] = (1/48) * sum_j sum_c had_k[k,j] * H64[m,c] * X[j,c]
with H64 the natural-order Sylvester Hadamard (symmetric).

Device scheme (per 6-token "group"):
  mm1 (data-stationary, one per group):
       lhsT = Xg[(t3,j)=108 part, (trip2,c)=128 free]
       rhs  = W36 = blockdiag(had_k.T x3)/48 [108,108]
       out  = Z[(trip2,c)=128, (t3,k)=108]  (PSUM fp32)
  mm2 (weights-stationary, one per QUAD of 4 groups):
       lhsT = W64 = blockdiag(H64 x2) [128,128]  (reloaded rarely)
       rhs  = Z quad [(trip2,c)=128 part, (4g,t3,k)=432 free]  (fp16 SBUF)
       out  = Y^T [(trip2,m)=128 part, (4g,t3,k)=432 free]  (PSUM, one bank)
  The transposed Y layout is free: the host un-permutes whatever layout
  the kernel stores.

HBM layout: the HOST pre-permutes x into the exact SBUF tile layout
(fp16), so every DMA is a fully contiguous 442 KB transfer with 4 KB
per-partition descriptors (a scatter-AP kernel runs at ~80-125 GB/s due
to 128/256-byte descriptors; contiguous runs near HBM rate).  The host
un-permutes the fp16 output back to token-major fp32.

Per core: 4096 tokens padded to 4128 = 43 superblocks x 16 groups x 6.
PSUM->SBUF copies batch 8 groups (one "oct"): z-copy on DVE, y-copy on
ACT.  The PE program is software-pipelined 2 octs deep (mm1 of oct i+2
before mm2 of oct i) so copy latency stays off the PE critical path.
DMA-completion semaphores are per buffer slot: the 16 SDMA engines'
increments from back-to-back DMAs interleave, so a shared counter
cannot prove any single transfer finished.
"""

import numpy as np

D = 2304
NTOK = 4096            # real tokens per core
NCORES = 8
GP_SB = 16             # groups per superblock (one DMA)
OCT = 8                # groups per PSUM batch
OPS = GP_SB // OCT     # octs per superblock = 2
NSB = 43               # superblocks per core
NGRP = NSB * GP_SB     # 688 groups = 4128 tokens (32 pad)
NTOKP = NGRP * 6       # 4128
FREE_SB = GP_SB * 128  # x elems per partition per superblock
YFREE = 108            # y free elems per group per partition
YFREE_SB = GP_SB * YFREE


def _h64():
    m, c = np.meshgrid(np.arange(64), np.arange(64), indexing="ij")
    bits = np.zeros((64, 64), np.int64)
    v = m & c
    for _ in range(6):
        bits += v & 1
        v >>= 1
    return np.where(bits % 2 == 0, 1.0, -1.0).astype(np.float32)


def _build_program(w36_np, w64_np):
    from contextlib import ExitStack
    import concourse.bass as bass
    import concourse.mybir as mybir
    from concourse.bass_types import AP

    nc = bass.Bass()
    x = nc.dram_tensor("x", [NSB * 108, FREE_SB], mybir.dt.float16,
                       kind="ExternalInput")
    y = nc.dram_tensor("y", [NSB * 128, YFREE_SB], mybir.dt.float16,
                       kind="ExternalOutput")
    w36_d = nc.inline_tensor(w36_np, name="w36")
    w64_d = nc.inline_tensor(w64_np, name="w64")

    NOCT = NSB * OPS  # 86

    # DMA "units" of two superblocks (884 KB): each transfer occupies its
    # queue ~(transfer + ~1.7 us completion receipt), so per-queue DMA
    # count — not bytes — was the limiter at one superblock per DMA.
    USB = 4                        # superblocks per DMA unit
    NU = (NSB + USB - 1) // USB    # 11 units; last unit is 3 sbs
    def u_sbs(u):
        return min(USB, NSB - USB * u)

    def u_octs(u):
        return OPS * u_sbs(u)

    def x_ap(u):
        return AP(tensor=x, offset=USB * u * 108 * FREE_SB,
                  ap=[[FREE_SB, 108], [108 * FREE_SB, u_sbs(u)], [1, FREE_SB]])

    def y_ap(u):
        return AP(tensor=y, offset=USB * u * 128 * YFREE_SB,
                  ap=[[YFREE_SB, 128], [128 * YFREE_SB, u_sbs(u)], [1, YFREE_SB]])

    with ExitStack() as ctx:
        w36 = ctx.enter_context(nc.sbuf_tensor("w36sb", [108, 108], mybir.dt.float16))
        w64 = ctx.enter_context(nc.sbuf_tensor("w64sb", [128, 128], mybir.dt.float16))
        xt = [ctx.enter_context(nc.sbuf_tensor(f"xt{i}", [108, 4 * GP_SB, 128], mybir.dt.float16)) for i in range(3)]
        yt = [ctx.enter_context(nc.sbuf_tensor(f"yt{i}", [128, 4 * GP_SB, YFREE], mybir.dt.float16)) for i in range(3)]
        zsb = [ctx.enter_context(nc.sbuf_tensor(f"zsb{i}", [128, OCT, YFREE], mybir.dt.float16)) for i in range(3)]
        zps = [ctx.enter_context(nc.psum_tensor(f"zps{i}", [128, OCT, 128], mybir.dt.float32)) for i in range(2)]
        yps = [ctx.enter_context(nc.psum_tensor(f"yps{i}", [128, OCT // 4, 512], mybir.dt.float32)) for i in range(2)]
        s_in = [ctx.enter_context(nc.semaphore(name=f"s_in{i}")) for i in range(3)]
        s_out = [ctx.enter_context(nc.semaphore(name=f"s_out{i}")) for i in range(3)]
        s_pe1 = ctx.enter_context(nc.semaphore())
        s_zc = ctx.enter_context(nc.semaphore())
        s_pe2 = ctx.enter_context(nc.semaphore())
        s_yc = ctx.enter_context(nc.semaphore())
        s_w = ctx.enter_context(nc.semaphore())
        blk = ctx.enter_context(nc.Block())

        # Loads and stores each alternate between the sync (HWDGE) and
        # gpsimd (SWDGE) queues; each queue's store trails its load by two
        # units so the store's semaphore wait never delays a ready load.
        OPU = OPS * USB  # octs per full unit = 8; s_pe1 counts half-octs (2/oct)

        def emit_load(s, u):
            if u >= 3:  # xt[u%3] reuse: mm1 octs of unit u-3 done
                s.wait_ge(s_pe1, 2 * OPU * (u - 2))
            ng = u_sbs(u) * GP_SB
            s.dma_start(xt[u % 3][:, 0:ng, :],
                        x_ap(u)).then_inc(s_in[u % 3], 16)

        def emit_store(s, u):
            s.wait_ge(s_yc, min(OPU * (u + 1), NOCT))
            ng = u_sbs(u) * GP_SB
            s.dma_start(y_ap(u),
                        yt[u % 3][:, 0:ng, :]).then_inc(s_out[u % 3], 16)

        @blk.sync
        def _(s):
            for u in range(0, NU, 2):
                emit_load(s, u)
                if u >= 2:
                    emit_store(s, u - 2)
            for u in (NU - 2, NU - 1):
                if u >= 0 and u % 2 == 0:
                    emit_store(s, u)

        def mm1(oi):
            u = oi // OPU
            g0 = (oi % OPU) * OCT
            for q in range(OCT):
                i = nc.tensor.matmul(zps[oi % 2][:, q, 0:108],
                                     xt[u % 3][:, g0 + q, :], w36[:, :],
                                     start=(q % 4 == 0), stop=(q % 4 == 3))
                if q == 3:
                    i.then_inc(s_pe1, 1)  # first half done: z-copy can start
            i.then_inc(s_pe1, 1)

        @blk.tensor
        def _(t):
            t.wait_ge(s_w, 32)
            t.wait_ge(s_in[0], 16)
            mm1(0)
            mm1(1)
            for oi in range(NOCT):
                # 2-deep software pipeline: mm1 of oct oi+2 before mm2 of
                # oct oi so the DVE z-copy of oct oi is hidden behind a
                # whole oct of PE work.  mm1(oi+2) (zps reuse) and mm2(oi)
                # (zsb ready) gate on the same z-copy(oi) event, so two
                # zps buffers suffice.
                if oi + 2 < NOCT:
                    u2 = (oi + 2) // OPU
                    if (oi + 2) % OPU == 0:
                        t.wait_ge(s_in[u2 % 3], 16 * (u2 // 3 + 1))
                    t.wait_ge(s_zc, 2 * oi + 2)  # zps[(oi+2)%2] freed by zc(oi)
                    mm1(oi + 2)
                if oi >= 2:                # yps[oi%2] freed by y-copy of oi-2
                    t.wait_ge(s_yc, oi - 1)
                for qq in range(OCT // 4):  # one wide matmul per 4 groups
                    # wait only for this half's z-copy
                    t.wait_ge(s_zc, 2 * oi + qq + 1)
                    i = nc.tensor.matmul(yps[oi % 2][:, qq, 0:432],
                                         w64[:, :],
                                         zsb[oi % 3][:, 4 * qq:4 * qq + 4, :],
                                         start=True, stop=True)
                i.then_inc(s_pe2, 1)

        @blk.vector
        def _(v):
            for oi in range(NOCT):
                for h in range(2):
                    v.wait_ge(s_pe1, 2 * oi + h + 1)
                    if oi >= 3 and h == 0:  # zsb[oi%3] consumed by mm2 of oi-3
                        v.wait_ge(s_pe2, oi - 2)
                    nc.vector.tensor_copy(
                        zsb[oi % 3][:, 4 * h:4 * h + 4, :],
                        zps[oi % 2][:, 4 * h:4 * h + 4, 0:108]).then_inc(s_zc, 1)

        @blk.scalar
        def _(a):
            for oi in range(NOCT):
                u = oi // OPU
                o = oi % OPU
                a.wait_ge(s_pe2, oi + 1)
                if u >= 3 and o == 0:  # yt[u%3] freed by store of unit u-3
                    a.wait_ge(s_out[u % 3], 16 * (u // 3))
                nc.scalar.copy(yt[u % 3][:, o * OCT:(o + 1) * OCT, :],
                               yps[oi % 2][:, :, 0:432]).then_inc(s_yc, 1)

        @blk.gpsimd
        def _(g):
            g.dma_start(w36[:, :], w36_d[:, :]).then_inc(s_w, 16)
            g.dma_start(w64[:, :], w64_d[:, :]).then_inc(s_w, 16)
            for u in range(1, NU, 2):
                emit_load(g, u)
                if u >= 2:
                    emit_store(g, u - 2)
            for u in (NU - 2, NU - 1):
                if u >= 0 and u % 2 == 1:
                    emit_store(g, u)
    return nc


def _permute_x(x):
    """[C, NTOK, D] fp32 -> [C, NSB*108, FREE_SB] fp16 in tile layout."""
    xc = np.zeros((NCORES, NTOKP, D), np.float16)
    xc[:, :NTOK, :] = x
    xv = xc.reshape(NCORES, NSB, GP_SB, 2, 3, 36, 64)
    xr = np.ascontiguousarray(xv.transpose(0, 1, 4, 5, 2, 3, 6))
    return xr.reshape(NCORES, NSB * 108, FREE_SB)


def _unpermute_y(yr):
    """[C, NSB*128, YFREE_SB] fp16 (transposed tile layout) ->
    [C, NTOK, D] fp32."""
    yv = yr.reshape(NCORES, NSB, 2, 64, GP_SB, 3, 36)
    out32 = np.empty((NCORES, NTOKP, D), np.float32)
    out32.reshape(NCORES, NSB, GP_SB, 2, 3, 36, 64)[...] = \
        yv.transpose(0, 1, 4, 2, 5, 6, 3)
    return np.ascontiguousarray(out32[:, :NTOK])


_CACHED = {}
_LAST_RES = None


def _run(x, had_k, trace=False):
    global _LAST_RES
    from concourse.bass_utils import run_bass_kernel_spmd

    x = np.asarray(x, dtype=np.float32)
    had_k = np.asarray(had_k, dtype=np.float32)

    h64 = _h64()
    w36_np = np.ascontiguousarray(
        (np.kron(np.eye(3, dtype=np.float32), had_k.T) / 48.0).astype(np.float16))
    w64_np = np.ascontiguousarray(
        np.kron(np.eye(2, dtype=np.float32), h64).astype(np.float16))

    key = w36_np.tobytes()
    if key not in _CACHED:
        _CACHED[key] = _build_program(w36_np, w64_np)
    nc = _CACHED[key]

    xr = _permute_x(x.reshape(NCORES, NTOK, D))
    in_maps = [{"x": xr[i]} for i in range(NCORES)]

    res = run_bass_kernel_spmd(nc, in_maps, core_ids=list(range(NCORES)),
                               trace=trace)
    _LAST_RES = res

    yr = np.stack([r["y"] for r in res.results])
    return _unpermute_y(yr).reshape(x.shape)


def kernel(x, had_k):
    return _run(x, had_k)


# revision 31
# speedup vs baseline: 1.0282x; 1.0043x over previous
"""Hadamard transform kernel for Trainium2 (8 NeuronCores, SPMD data parallel).

y = (1/48) * (H36 (x) H64) @ x_row  per token row, x: (4, 8192, 2304) fp32.

Math: view each row as X[j=36, c=64] (row-major).  Then
    y[k*64+m] = (1/48) * sum_j sum_c had_k[k,j] * H64[m,c] * X[j,c]
with H64 the natural-order Sylvester Hadamard (symmetric).

Device scheme (per 6-token "group"):
  mm1 (data-stationary, one per group):
       lhsT = Xg[(t3,j)=108 part, (trip2,c)=128 free]
       rhs  = W36 = blockdiag(had_k.T x3)/48 [108,108]
       out  = Z[(trip2,c)=128, (t3,k)=108]  (PSUM fp32)
  mm2 (weights-stationary, one per QUAD of 4 groups):
       lhsT = W64 = blockdiag(H64 x2) [128,128]  (reloaded rarely)
       rhs  = Z quad [(trip2,c)=128 part, (4g,t3,k)=432 free]  (fp16 SBUF)
       out  = Y^T [(trip2,m)=128 part, (4g,t3,k)=432 free]  (PSUM, one bank)
  The transposed Y layout is free: the host un-permutes whatever layout
  the kernel stores.

HBM layout: the HOST pre-permutes x into the exact SBUF tile layout
(fp16), so every DMA is a fully contiguous 442 KB transfer with 4 KB
per-partition descriptors (a scatter-AP kernel runs at ~80-125 GB/s due
to 128/256-byte descriptors; contiguous runs near HBM rate).  The host
un-permutes the fp16 output back to token-major fp32.

Per core: 4096 tokens padded to 4128 = 43 superblocks x 16 groups x 6.
PSUM->SBUF copies batch 8 groups (one "oct"): z-copy on DVE, y-copy on
ACT.  The PE program is software-pipelined 2 octs deep (mm1 of oct i+2
before mm2 of oct i) so copy latency stays off the PE critical path.
DMA-completion semaphores are per buffer slot: the 16 SDMA engines'
increments from back-to-back DMAs interleave, so a shared counter
cannot prove any single transfer finished.
"""

import numpy as np

D = 2304
NTOK = 4096            # real tokens per core
NCORES = 8
GP_SB = 16             # groups per superblock (one DMA)
OCT = 8                # groups per PSUM batch
OPS = GP_SB // OCT     # octs per superblock = 2
NSB = 43               # superblocks per core
NGRP = NSB * GP_SB     # 688 groups = 4128 tokens (32 pad)
NTOKP = NGRP * 6       # 4128
FREE_SB = GP_SB * 128  # x elems per partition per superblock
YFREE = 108            # y free elems per group per partition
YFREE_SB = GP_SB * YFREE


def _h64():
    m, c = np.meshgrid(np.arange(64), np.arange(64), indexing="ij")
    bits = np.zeros((64, 64), np.int64)
    v = m & c
    for _ in range(6):
        bits += v & 1
        v >>= 1
    return np.where(bits % 2 == 0, 1.0, -1.0).astype(np.float32)


def _build_program(w36_np, w64_np):
    from contextlib import ExitStack
    import concourse.bass as bass
    import concourse.mybir as mybir
    from concourse.bass_types import AP

    nc = bass.Bass()
    x = nc.dram_tensor("x", [NSB * 108, FREE_SB], mybir.dt.float16,
                       kind="ExternalInput")
    y = nc.dram_tensor("y", [NSB * 128, YFREE_SB], mybir.dt.float16,
                       kind="ExternalOutput")
    w36_d = nc.inline_tensor(w36_np, name="w36")
    w64_d = nc.inline_tensor(w64_np, name="w64")

    NOCT = NSB * OPS  # 86

    # DMA "units" of two superblocks (884 KB): each transfer occupies its
    # queue ~(transfer + ~1.7 us completion receipt), so per-queue DMA
    # count — not bytes — was the limiter at one superblock per DMA.
    USB = 4                        # superblocks per DMA unit
    NU = (NSB + USB - 1) // USB    # 11 units; last unit is 3 sbs
    def u_sbs(u):
        return min(USB, NSB - USB * u)

    def u_octs(u):
        return OPS * u_sbs(u)

    def x_ap(u):
        return AP(tensor=x, offset=USB * u * 108 * FREE_SB,
                  ap=[[FREE_SB, 108], [108 * FREE_SB, u_sbs(u)], [1, FREE_SB]])

    def y_ap(u):
        return AP(tensor=y, offset=USB * u * 128 * YFREE_SB,
                  ap=[[YFREE_SB, 128], [128 * YFREE_SB, u_sbs(u)], [1, YFREE_SB]])

    with ExitStack() as ctx:
        w36 = ctx.enter_context(nc.sbuf_tensor("w36sb", [108, 108], mybir.dt.float16))
        w64 = ctx.enter_context(nc.sbuf_tensor("w64sb", [128, 128], mybir.dt.float16))
        xt = [ctx.enter_context(nc.sbuf_tensor(f"xt{i}", [108, 4 * GP_SB, 128], mybir.dt.float16)) for i in range(3)]
        yt = [ctx.enter_context(nc.sbuf_tensor(f"yt{i}", [128, 4 * GP_SB, YFREE], mybir.dt.float16)) for i in range(3)]
        zsb = [ctx.enter_context(nc.sbuf_tensor(f"zsb{i}", [128, OCT, YFREE], mybir.dt.float16)) for i in range(3)]
        zps = [ctx.enter_context(nc.psum_tensor(f"zps{i}", [128, OCT, 128], mybir.dt.float32)) for i in range(2)]
        yps = [ctx.enter_context(nc.psum_tensor(f"yps{i}", [128, OCT // 4, 512], mybir.dt.float32)) for i in range(2)]
        s_in = [ctx.enter_context(nc.semaphore(name=f"s_in{i}")) for i in range(3)]
        s_out = [ctx.enter_context(nc.semaphore(name=f"s_out{i}")) for i in range(3)]
        s_pe1 = ctx.enter_context(nc.semaphore())
        s_zc = ctx.enter_context(nc.semaphore())
        s_pe2 = ctx.enter_context(nc.semaphore())
        s_yc = ctx.enter_context(nc.semaphore())
        s_w = ctx.enter_context(nc.semaphore())
        blk = ctx.enter_context(nc.Block())

        # Loads and stores each alternate between the sync (HWDGE) and
        # gpsimd (SWDGE) queues; each queue's store trails its load by two
        # units so the store's semaphore wait never delays a ready load.
        OPU = OPS * USB  # octs per full unit = 8; s_pe1 counts half-octs (2/oct)

        def emit_load(s, u):
            if u >= 3:  # xt[u%3] reuse: mm1 octs of unit u-3 done
                s.wait_ge(s_pe1, 2 * OPU * (u - 2))
            ng = u_sbs(u) * GP_SB
            s.dma_start(xt[u % 3][:, 0:ng, :],
                        x_ap(u)).then_inc(s_in[u % 3], 16)

        def emit_store(s, u):
            s.wait_ge(s_yc, min(OPU * (u + 1), NOCT))
            ng = u_sbs(u) * GP_SB
            s.dma_start(y_ap(u), yt[u % 3][:, 0:ng, :],
                        single_packet=True).then_inc(s_out[u % 3], 16)

        @blk.sync
        def _(s):
            for u in range(0, NU, 2):
                emit_load(s, u)
                if u >= 2:
                    emit_store(s, u - 2)
            for u in (NU - 2, NU - 1):
                if u >= 0 and u % 2 == 0:
                    emit_store(s, u)

        def mm1(oi):
            u = oi // OPU
            g0 = (oi % OPU) * OCT
            for q in range(OCT):
                i = nc.tensor.matmul(zps[oi % 2][:, q, 0:108],
                                     xt[u % 3][:, g0 + q, :], w36[:, :],
                                     start=(q % 4 == 0), stop=(q % 4 == 3))
                if q == 3:
                    i.then_inc(s_pe1, 1)  # first half done: z-copy can start
            i.then_inc(s_pe1, 1)

        @blk.tensor
        def _(t):
            t.wait_ge(s_w, 32)
            t.wait_ge(s_in[0], 16)
            mm1(0)
            mm1(1)
            for oi in range(NOCT):
                # 2-deep software pipeline: mm1 of oct oi+2 before mm2 of
                # oct oi so the DVE z-copy of oct oi is hidden behind a
                # whole oct of PE work.  mm1(oi+2) (zps reuse) and mm2(oi)
                # (zsb ready) gate on the same z-copy(oi) event, so two
                # zps buffers suffice.
                if oi + 2 < NOCT:
                    u2 = (oi + 2) // OPU
                    if (oi + 2) % OPU == 0:
                        t.wait_ge(s_in[u2 % 3], 16 * (u2 // 3 + 1))
                    t.wait_ge(s_zc, 2 * oi + 2)  # zps[(oi+2)%2] freed by zc(oi)
                    mm1(oi + 2)
                if oi >= 2:                # yps[oi%2] freed by y-copy of oi-2
                    t.wait_ge(s_yc, oi - 1)
                for qq in range(OCT // 4):  # one wide matmul per 4 groups
                    # wait only for this half's z-copy
                    t.wait_ge(s_zc, 2 * oi + qq + 1)
                    i = nc.tensor.matmul(yps[oi % 2][:, qq, 0:432],
                                         w64[:, :],
                                         zsb[oi % 3][:, 4 * qq:4 * qq + 4, :],
                                         start=True, stop=True)
                i.then_inc(s_pe2, 1)

        @blk.vector
        def _(v):
            for oi in range(NOCT):
                for h in range(2):
                    v.wait_ge(s_pe1, 2 * oi + h + 1)
                    if oi >= 3 and h == 0:  # zsb[oi%3] consumed by mm2 of oi-3
                        v.wait_ge(s_pe2, oi - 2)
                    nc.vector.tensor_copy(
                        zsb[oi % 3][:, 4 * h:4 * h + 4, :],
                        zps[oi % 2][:, 4 * h:4 * h + 4, 0:108]).then_inc(s_zc, 1)

        @blk.scalar
        def _(a):
            for oi in range(NOCT):
                u = oi // OPU
                o = oi % OPU
                a.wait_ge(s_pe2, oi + 1)
                if u >= 3 and o == 0:  # yt[u%3] freed by store of unit u-3
                    a.wait_ge(s_out[u % 3], 16 * (u // 3))
                nc.scalar.copy(yt[u % 3][:, o * OCT:(o + 1) * OCT, :],
                               yps[oi % 2][:, :, 0:432]).then_inc(s_yc, 1)

        @blk.gpsimd
        def _(g):
            g.dma_start(w36[:, :], w36_d[:, :]).then_inc(s_w, 16)
            g.dma_start(w64[:, :], w64_d[:, :]).then_inc(s_w, 16)
            for u in range(1, NU, 2):
                emit_load(g, u)
                if u >= 2:
                    emit_store(g, u - 2)
            for u in (NU - 2, NU - 1):
                if u >= 0 and u % 2 == 1:
                    emit_store(g, u)
    return nc


def _permute_x(x):
    """[C, NTOK, D] fp32 -> [C, NSB*108, FREE_SB] fp16 in tile layout."""
    xc = np.zeros((NCORES, NTOKP, D), np.float16)
    xc[:, :NTOK, :] = x
    xv = xc.reshape(NCORES, NSB, GP_SB, 2, 3, 36, 64)
    xr = np.ascontiguousarray(xv.transpose(0, 1, 4, 5, 2, 3, 6))
    return xr.reshape(NCORES, NSB * 108, FREE_SB)


def _unpermute_y(yr):
    """[C, NSB*128, YFREE_SB] fp16 (transposed tile layout) ->
    [C, NTOK, D] fp32."""
    yv = yr.reshape(NCORES, NSB, 2, 64, GP_SB, 3, 36)
    out32 = np.empty((NCORES, NTOKP, D), np.float32)
    out32.reshape(NCORES, NSB, GP_SB, 2, 3, 36, 64)[...] = \
        yv.transpose(0, 1, 4, 2, 5, 6, 3)
    return np.ascontiguousarray(out32[:, :NTOK])


_CACHED = {}
_LAST_RES = None


def _run(x, had_k, trace=False):
    global _LAST_RES
    from concourse.bass_utils import run_bass_kernel_spmd

    x = np.asarray(x, dtype=np.float32)
    had_k = np.asarray(had_k, dtype=np.float32)

    h64 = _h64()
    w36_np = np.ascontiguousarray(
        (np.kron(np.eye(3, dtype=np.float32), had_k.T) / 48.0).astype(np.float16))
    w64_np = np.ascontiguousarray(
        np.kron(np.eye(2, dtype=np.float32), h64).astype(np.float16))

    key = w36_np.tobytes()
    if key not in _CACHED:
        _CACHED[key] = _build_program(w36_np, w64_np)
    nc = _CACHED[key]

    xr = _permute_x(x.reshape(NCORES, NTOK, D))
    in_maps = [{"x": xr[i]} for i in range(NCORES)]

    res = run_bass_kernel_spmd(nc, in_maps, core_ids=list(range(NCORES)),
                               trace=trace)
    _LAST_RES = res

    yr = np.stack([r["y"] for r in res.results])
    return _unpermute_y(yr).reshape(x.shape)


def kernel(x, had_k):
    return _run(x, had_k)
